# revision 2
# baseline (speedup 1.0000x reference)
"""Trainium2 Bass kernel for the CoAttnLayer problem.

Computes, per example b (B=16, all dims 1024):
    P   = E @ W^T                      (proj)
    S   = P @ Src^T                    (sim, (e, l))
    edit_w  = softmax_l(S + srcmask)   -> edit_ctx = edit_w @ Src
    src_w   = softmax_e(S + editmask)  -> src_ctx  = src_w^T @ E

Sharding: data-parallel over batch, 2 examples per core on 8 cores; W^T is
loaded once per core. All heavy matmuls run in float32r (fp32 operands
truncated by the PE, full fp32 PSUM accumulation).

Design notes (vs a straightforward per-path implementation):

 *  The src path (softmax over e, the PARTITION axis of sim) does not build
    a transposed S^T matrix. It reuses the edit path's eexp = exp(S - rowmax)
    directly as the matmul stationary operand and repairs the row-dependent
    shift with a per-example scalar shift c:
        src_w[:, l] ~ e01[e] * exp(sim[e,l] - c) = eexp[e,l] * w[e],
        w[e] = e01[e] * exp((rowmax[e] - c) * e01[e]),
        c    = (max over valid e of rowmax) - 30.
    w is folded into the moving operand (E's rows are scaled by w in place),
    so the context matmul needs no transposes at all. The normalizer
    Z[l] = sum_e w[e]*eexp[e,l] is computed for all l at once by M=1
    matmuls (w column stationary, eexp moving) and transposed back to
    per-partition columns in 2-wide strips. The -30 in c keeps both w
    (<= e^30) and Z (>= ~e^-48 for this data) far from fp32 range limits;
    the shift cancels exactly in the normalization.

 *  The edit path's softmax reduction is along the free axis: rowmax via DVE,
    exp via the scalar engine writing f32r, whose accum_out also yields the
    Z sum over the region that is fully valid for EVERY example sharing the
    compiled program ([0, (LVMIN-1)*128)); the remaining partially-valid
    region gets a masked DVE mul+reduce against a broadcast 0/1 row. The
    exp weights are transposed on the PE (128x128 f32r transposes) into the
    stationary layout of the context matmul; masked l-tiles are skipped and
    the partial boundary tile is exact because Src's masked rows are zeroed
    on the host.

 *  Masked-tile skipping with slot-uniform counts: the host pairs a
    low-edit-len/high-src-len example with the reverse on each core and
    compiles per-slot valid 128-tile counts (EV, LV, LVMIN) = max/min over
    the 8 examples mapped to that slot, so all cores share one program.
    Context matmuls/transposes cover only valid tiles.

 *  Phase 2 is software-pipelined: the sim matmuls of e-block mb+1 are
    issued between the sim and the transposes/context matmuls of block mb,
    so the PE never waits on the DVE->ACT softmax chain, and the PSUM plan
    (2x [P,1024] psmm + 4x [P,512] ps1 = 8 banks) keeps two sim blocks in
    flight. The src-shift chain is emitted before the last edit block and
    E-natural is prefetched during phase 2, hiding both at the example
    boundary. Initial W^T/E^T loads are interleaved so phase 1 starts on
    the first slices.
"""

import os
import sys

for _p in ("/opt/trn_rl_repo",):
    if os.path.isdir(_p) and _p not in sys.path:
        sys.path.insert(0, _p)

import numpy as np

import concourse.bass as bass
import concourse.tile as tile
from concourse import mybir
from concourse.bass import ts
from concourse.masks import make_identity
from concourse.bass_utils import run_bass_kernel_spmd

B = 16
L = 1024
NCORES = 8
PER_CORE = B // NCORES
P = 128
KB = L // P
N2 = 2
F32 = mybir.dt.float32
F32R = mybir.dt.float32r
AX = mybir.AxisListType.X
EXP = mybir.ActivationFunctionType.Exp
IDENT = mybir.ActivationFunctionType.Identity
MIN = mybir.AluOpType.min


def build_nc(slots=((6, 8, 7), (8, 6, 4)), fence=True):
    """slots: per-example-slot (EV, LV, LVMIN) valid 128-tile counts.
    LV = max valid src tiles among slot members, LVMIN = min; the region
    [(LVMIN-1)*128, LV*128) gets a masked Z sum (per-example 0/1 data),
    the region below it is fully valid for every member."""
    nc = bass.Bass("TRN2", target_bir_lowering=False)
    n = len(slots)

    et_d = nc.dram_tensor("et", (n, L, L), F32, kind="ExternalInput")      # E^T  (d, e)
    srct_d = nc.dram_tensor("srct", (n, L, L), F32, kind="ExternalInput")  # Src^T (s, l)
    srcz_d = nc.dram_tensor("srcz", (n, L, L), F32, kind="ExternalInput")  # Src, masked rows zeroed (l, s)
    en_d = nc.dram_tensor("en", (n, L, L), F32, kind="ExternalInput")      # E natural (e, d)
    wt_d = nc.dram_tensor("wt", (L, L), F32, kind="ExternalInput")         # W^T (d, s)
    bwids = [(lv - lvmin + 1) * P for (_, lv, lvmin) in slots]
    BW = max(bwids)
    s01b_d = nc.dram_tensor("s01b", (n, BW), F32, kind="ExternalInput")    # src validity 0/1, boundary region row
    e01_d = nc.dram_tensor("e01", (n, P, KB), F32, kind="ExternalInput")   # edit validity 0/1 [p, eb]
    ebig_d = nc.dram_tensor("ebig", (n, P, KB), F32, kind="ExternalInput")  # (1-e01)*1e30
    oe_d = nc.dram_tensor("oe", (n, L, L), F32, kind="ExternalOutput")     # edit_ctx
    os_d = nc.dram_tensor("osr", (n, L, L), F32, kind="ExternalOutput")    # src_ctx

    def load_split(sb_tile, dram, kslices=KB, eng=None):
        for k in range(kslices):
            (eng or nc.sync).dma_start(
                out=sb_tile[:, k, :],
                in_=dram[k * P : (k + 1) * P, :].bitcast(F32R),
            )

    with tile.TileContext(nc) as tc:
        with (
            tc.tile_pool(name="persist", bufs=1) as persist,
            tc.tile_pool(name="big", bufs=4) as big,
            tc.tile_pool(name="exps", bufs=1) as exps,
            tc.tile_pool(name="blk", bufs=2) as blk,
            tc.tile_pool(name="expt", bufs=1) as expt,
            tc.tile_pool(name="stats", bufs=7) as stats,
            tc.tile_pool(name="small", bufs=6) as small,
            tc.tile_pool(name="s01p", bufs=2) as s01p,
            tc.tile_pool(name="psmm", bufs=2, space="PSUM") as psmm,
            tc.tile_pool(name="ps1", bufs=4, space="PSUM") as ps1,
        ):
            identr = persist.tile([P, P], F32R, tag="identr")
            ident_tmp = blk.tile([P, 512], F32, tag="blk", name="ident_tmp")
            make_identity(nc, ident_tmp[:, :P])
            nc.vector.tensor_copy(identr, ident_tmp[:, :P])
            ones_r = persist.tile([1, P], F32R, tag="ones")
            ones_tmp = blk.tile([P, 512], F32, tag="blk", name="ones_tmp")
            nc.vector.memset(ones_tmp[0:1, :P], 1.0)
            nc.vector.tensor_copy(ones_r, ones_tmp[0:1, :P])
            wt_sb = persist.tile([P, KB, L], F32R, tag="wt")

            st = [dict() for _ in range(n)]

            def load_head(b, interleave_wt=False):
                d = st[b]
                d["et"] = big.tile([P, KB, L], F32R, tag="big", name=f"et_{b}")
                if interleave_wt:
                    for k in range(KB):
                        nc.sync.dma_start(
                            out=wt_sb[:, k, :],
                            in_=wt_d[k * P : (k + 1) * P, :].bitcast(F32R),
                        )
                        nc.sync.dma_start(
                            out=d["et"][:, k, :],
                            in_=et_d[b][k * P : (k + 1) * P, :].bitcast(F32R),
                        )
                else:
                    load_split(d["et"], et_d[b])

            def phase1(b):
                d = st[b]
                pt = big.tile([P, KB, L], F32R, tag="big", name=f"pt_{b}")
                for sb in range(KB):
                    ps = psmm.tile([P, L], F32, tag="psmm", name=f"p1_{b}_{sb}")
                    for k in range(KB):
                        for nt in range(N2):
                            nc.tensor.matmul(
                                ps[:, ts(nt, 512)],
                                wt_sb[:, k, ts(sb, P)],
                                d["et"][:, k, ts(nt, 512)],
                                start=(k == 0),
                                stop=(k == KB - 1),
                            )
                    nc.scalar.copy(pt[:, sb, :], ps)
                d["pt"] = pt
                d["et"] = None

            def load_mid(b, ev, lv, lvmin):
                d = st[b]
                d["srct"] = big.tile([P, KB, L], F32R, tag="big", name=f"srct_{b}")
                load_split(d["srct"], srct_d[b])
                d["srcz"] = big.tile([P, KB, L], F32R, tag="big", name=f"srcz_{b}")
                load_split(d["srcz"], srcz_d[b], kslices=lv)
                # src validity of the boundary region, broadcast across
                # partitions (host pre-slices it per slot)
                bw = (lv - lvmin + 1) * P
                d["s01b"] = s01p.tile([P, BW], F32, tag="s01b", name=f"s01b_{b}")
                src_ap = bass.AP(
                    tensor=s01b_d[b].tensor,
                    offset=s01b_d[b].offset,
                    ap=[[0, P]] + list(s01b_d[b, 0:bw].ap),
                )
                nc.gpsimd.dma_start(out=d["s01b"][:, 0:bw], in_=src_ap)
                d["e01"] = small.tile([P, KB], F32, tag="e01", name=f"e01_{b}")
                nc.sync.dma_start(out=d["e01"], in_=e01_d[b])
                d["ebig"] = small.tile([P, KB], F32, tag="ebig", name=f"ebig_{b}")
                nc.sync.dma_start(out=d["ebig"], in_=ebig_d[b])

            def sim_block(b, mb, ev, lv, lvmin):
                """sim row-block mb -> PSUM; rowmax; exp into expS with
                Z-accumulation over the fully-valid l range."""
                d = st[b]
                expS, nm_all = d["expS"], d["nm"]
                ps = psmm.tile([P, L], F32, tag="psmm", name=f"p2_{b}_{mb}")
                for k in range(KB):
                    for nt in range(N2):
                        nc.tensor.matmul(
                            ps[:, ts(nt, 512)],
                            d["pt"][:, k, ts(mb, P)],
                            d["srct"][:, k, ts(nt, 512)],
                            start=(k == 0),
                            stop=(k == KB - 1),
                        )
                nc.vector.reduce_max(nm_all[:, mb : mb + 1], ps, axis=AX, negate=True)
                bias = nm_all[:, mb : mb + 1]
                main_w = (lvmin - 1) * P
                bw = (lv - lvmin + 1) * P
                zmain = stats.tile([P, 1], F32, tag="zm", name=f"zm_{b}_{mb}")
                nc.scalar.activation(
                    expS[:, mb, 0:main_w], ps[:, 0:main_w], EXP,
                    bias=bias, accum_out=zmain,
                )
                # boundary region (mask varies per member; masked Z sum on DVE)
                nc.scalar.activation(
                    expS[:, mb, main_w : lv * P], ps[:, main_w : lv * P], EXP,
                    bias=bias,
                )
                if lv < KB:
                    # invalid region: needed by the src path only
                    nc.scalar.activation(
                        expS[:, mb, lv * P :], ps[:, lv * P :], EXP, bias=bias
                    )
                zb_t = blk.tile([P, 512], F32, tag="blk", name=f"zbt_{b}_{mb}")
                nc.vector.tensor_mul(
                    zb_t[:, 0:bw], expS[:, mb, main_w : lv * P], d["s01b"][:, 0:bw]
                )
                zbnd = stats.tile([P, 1], F32, tag="zb", name=f"zb_{b}_{mb}")
                nc.vector.reduce_sum(zbnd, zb_t[:, 0:bw], axis=AX)
                ze = stats.tile([P, 1], F32, tag="ze", name=f"ze_{b}_{mb}")
                nc.vector.tensor_add(ze, zmain, zbnd)
                iz = stats.tile([P, 1], F32, tag="iz", name=f"iz_{b}_{mb}")
                nc.vector.reciprocal(iz, ze)
                d["iz"][mb] = iz

            def edit_block(b, mb, ev, lv):
                """transpose eexp's valid l-tiles; edit context halves."""
                d = st[b]
                expS = d["expS"]
                eexpT = expt.tile([P, KB, P], F32R, tag="expt", name=f"eT_{b}_{mb}")
                for c0 in range(0, lv, 4):
                    cn = min(4, lv - c0)
                    tr = ps1.tile([P, 4, P], F32R, tag="ps1", name=f"tr_{b}_{mb}_{c0}")
                    for i in range(cn):
                        nc.tensor.transpose(
                            tr[:, i, :], expS[:, mb, ts(c0 + i, P)], identr
                        )
                    nc.vector.tensor_copy(eexpT[:, c0 : c0 + cn, :], tr[:, :cn, :])
                cps = [
                    ps1.tile([P, 512], F32, tag="ps1", name=f"ec_{b}_{mb}_{nt}")
                    for nt in range(N2)
                ]
                for k in range(lv):
                    for nt in range(N2):
                        nc.tensor.matmul(
                            cps[nt],
                            eexpT[:, k, :],
                            d["srcz"][:, k, ts(nt, 512)],
                            start=(k == 0),
                            stop=(k == lv - 1),
                        )
                for nt in range(N2):
                    oe_stage = blk.tile([P, 512], F32, tag="blk", name=f"oes_{b}_{mb}_{nt}")
                    nc.scalar.mul(oe_stage, cps[nt], mul=d["iz"][mb])
                    nc.sync.dma_start(
                        out=oe_d[b, mb * P : (mb + 1) * P, ts(nt, 512)], in_=oe_stage
                    )

            def phase2(b, ev, lv, lvmin):
                d = st[b]
                d["expS"] = exps.tile([P, KB, L], F32R, tag="expS", name=f"expS_{b}")
                d["nm"] = stats.tile([P, KB], F32, tag="nm", name=f"nm_{b}")
                d["iz"] = [None] * KB
                # preload E-natural now; its big-pool slot (freed by phase1)
                # lets the DMA overlap phase 2 compute
                en = big.tile([P, KB, L], F32R, tag="big", name=f"en_{b}")
                load_split(en, en_d[b], kslices=ev)
                d["en"] = en
                sim_block(b, 0, ev, lv, lvmin)
                for mb in range(1, KB):
                    sim_block(b, mb, ev, lv, lvmin)
                    edit_block(b, mb - 1, ev, lv)
                # the src-path shift chain only needs the rowmaxes; emit it
                # before the last edit block so its tiny-op latency hides
                # under the remaining PE work
                prep_src(b, ev, lv)
                edit_block(b, KB - 1, ev, lv)
                d["pt"] = None
                d["srct"] = None
                d["srcz"] = None

            def prep_src(b, ev, lv):
                d = st[b]
                nm_v = stats.tile([P, KB], F32, tag="nmv", name=f"nmv_{b}")
                nc.vector.tensor_add(nm_v, d["nm"], d["ebig"])
                nm_min = stats.tile([P, 1], F32R, tag="nmm", name=f"nmm_{b}")
                nc.vector.tensor_reduce(nm_min, nm_v, axis=AX, op=MIN)
                nm_row = ps1.tile([1, P], F32R, tag="ps1", name=f"nmr_{b}")
                nc.tensor.transpose(nm_row, nm_min, identr)
                gn = stats.tile([1, 1], F32, tag="gn", name=f"gn_{b}")
                nc.vector.tensor_reduce(gn, nm_row, axis=AX, op=MIN)
                # fp32r matmuls need an 8-byte-granular destination -> N=2
                gn30 = stats.tile([1, 2], F32R, tag="gn30", name=f"gn30_{b}")
                nc.vector.tensor_scalar_add(gn30[:, 0:1], gn, 30.0)
                nc.vector.tensor_scalar_add(gn30[:, 1:2], gn, 30.0)
                gps = ps1.tile([P, 2], F32, tag="ps1", name=f"gps_{b}")
                nc.tensor.matmul(gps, ones_r, gn30, start=True, stop=True)
                gcol = stats.tile([P, 1], F32, tag="gcol", name=f"gcol_{b}")
                nc.vector.tensor_copy(gcol, gps[:, 0:1])
                warg = stats.tile([P, KB], F32, tag="warg", name=f"warg_{b}")
                nc.scalar.activation(warg, d["nm"], IDENT, bias=gcol, scale=-1.0)
                nc.vector.tensor_mul(warg, warg, d["e01"])
                w_all = small.tile([P, KB], F32, tag="w", name=f"w_{b}")
                nc.scalar.activation(w_all, warg, EXP)
                nc.vector.tensor_mul(w_all, w_all, d["e01"])
                wr2 = small.tile([P, KB, 2], F32R, tag="wr", name=f"wr_{b}")
                nc.vector.tensor_copy(wr2[:, :, 0:1], w_all)
                nc.vector.tensor_copy(wr2[:, :, 1:2], w_all)
                en = d["en"]
                for k in range(ev):
                    nc.vector.tensor_scalar_mul(en[:, k, :], en[:, k, :], w_all[:, k : k + 1])
                d["wr2"] = wr2

            def phase3(b, ev, lv):
                d = st[b]
                # Z over all l at once: zrow[0, l] = sum_e w[e] * eexp[e, l],
                # via M=1 matmuls (w column stationary, eexp rows moving)
                zps = [
                    ps1.tile([1, 512], F32, tag="ps1", name=f"zr_{b}_{nt}")
                    for nt in range(N2)
                ]
                for k in range(ev):
                    st_, sp_ = (k == 0), (k == ev - 1)
                    for nt in range(N2):
                        nc.tensor.matmul(
                            zps[nt],
                            d["wr2"][:, k, 0:1],
                            d["expS"][:, k, ts(nt, 512)],
                            start=st_,
                            stop=sp_,
                        )
                zrow = expt.tile([2, L], F32R, tag="expt", name=f"zrow_{b}")
                for nt in range(N2):
                    nc.vector.tensor_copy(zrow[0:1, ts(nt, 512)], zps[nt])
                    nc.sync.dma_start(out=zrow[1:2, ts(nt, 512)], in_=zrow[0:1, ts(nt, 512)])
                for lb in range(KB):
                    cps = psmm.tile([P, L], F32, tag="psmm", name=f"sc_{b}_{lb}")
                    for k in range(ev):
                        st_, sp_ = (k == 0), (k == ev - 1)
                        for nt in range(N2):
                            nc.tensor.matmul(
                                cps[:, ts(nt, 512)],
                                d["expS"][:, k, ts(lb, P)],
                                d["en"][:, k, ts(nt, 512)],
                                start=st_,
                                stop=sp_,
                            )
                    zt = ps1.tile([P, 2], F32R, tag="ps1", name=f"zt_{b}_{lb}")
                    nc.tensor.transpose(zt, zrow[:, ts(lb, P)], identr[0:2, 0:2])
                    izl = stats.tile([P, 1], F32, tag="iz", name=f"izl_{b}_{lb}")
                    nc.vector.reciprocal(izl, zt[:, 0:1])
                    for nt in range(N2):
                        os_stage = blk.tile([P, 512], F32, tag="blk", name=f"oss_{b}_{lb}_{nt}")
                        nc.vector.tensor_scalar_mul(os_stage, cps[:, ts(nt, 512)], izl)
                        nc.sync.dma_start(
                            out=os_d[b, lb * P : (lb + 1) * P, ts(nt, 512)], in_=os_stage
                        )
                d["expS"] = None
                d["en"] = None

            load_head(0, interleave_wt=True)
            phase1(0)
            for b in range(n):
                ev, lv, lvmin = slots[b]
                load_mid(b, ev, lv, lvmin)
                phase2(b, ev, lv, lvmin)
                if b + 1 < n:
                    load_head(b + 1)
                    phase1(b + 1)
                phase3(b, ev, lv)

    if fence:
        _fence_matmul_waits(nc)
    return nc


def _fence_matmul_waits(nc):
    """walrus can attach at most one sync wait to the LDWEIGHTS half of a
    self-loading fp32/fp32r matmul. Move every multi-wait Matmult's waits
    onto a PE no-op fence inserted right before it."""
    f = nc.m.functions[0]
    moved = 0
    for blk in f.blocks:
        out = []
        for inst in blk.instructions:
            si = getattr(inst, "sync_info", None)
            if si is not None and len(si.on_wait) > 1:
                for j, w in enumerate(si.on_wait):
                    nop = mybir.InstNoOp(name=f"{inst.name}-wf{j}", ins=[], outs=[])
                    nop.engine = inst.engine
                    nop.sync_info = mybir.SyncInfo(on_wait=[w], on_update=[])
                    out.append(nop)
                inst.sync_info = mybir.SyncInfo(on_wait=[], on_update=list(si.on_update))
                moved += 1
            out.append(inst)
        blk.instructions = out
    return moved


_NC_CACHE = {}


def get_nc(slots=((6, 8, 7), (8, 6, 4))):
    key = tuple(slots)
    if key not in _NC_CACHE:
        _NC_CACHE[key] = build_nc(key)
    return _NC_CACHE[key]


def plan_assignment(em, sm):
    def vtiles(mask):
        valid = np.flatnonzero(mask == 0)
        hi = int(valid[-1]) if valid.size else 0
        return (hi // P) + 1

    def first_masked_tile(mask):
        bad = np.flatnonzero(mask != 0)
        first = int(bad[0]) if bad.size else L
        return first // P  # tiles strictly below this index are fully valid

    ev = np.array([vtiles(em[b]) for b in range(B)])
    lv = np.array([vtiles(sm[b]) for b in range(B)])
    fm = np.array([first_masked_tile(sm[b]) for b in range(B)])
    order = np.argsort(ev, kind="stable")
    n_slots = PER_CORE
    assignment = [
        [int(order[s * NCORES + c]) for s in range(n_slots)] for c in range(NCORES)
    ]
    slots = []
    for s in range(n_slots):
        members = [int(order[s * NCORES + c]) for c in range(NCORES)]
        lvmax = int(lv[members].max())
        # main region must be fully valid for every member
        lvmin = int(min(lv[members].min(), fm[members].min() + 1))
        lvmin = max(1, min(lvmin, lvmax))
        slots.append((int(ev[members].max()), lvmax, lvmin))
    return assignment, tuple(slots)


def make_in_maps(E, S, em, sm, W32, assignment, slots=None):
    if slots is None:
        _, slots = plan_assignment(em, sm)
    bwids = [(lv - lvmin + 1) * P for (_, lv, lvmin) in slots]
    BW = max(bwids)
    wt = np.ascontiguousarray(W32.T)
    in_maps = []
    for c in range(NCORES):
        bs = assignment[c]
        et = np.stack([np.ascontiguousarray(E[b].T) for b in bs])
        srct = np.stack([np.ascontiguousarray(S[b].T) for b in bs])
        srcz = np.stack(
            [S[b] * (1.0 - sm[b])[:, None].astype(np.float32) for b in bs]
        )
        en = np.ascontiguousarray(E[bs])
        s01b = np.zeros((len(bs), BW), np.float32)
        for si, b in enumerate(bs):
            _, lv_s, lvmin_s = slots[si]
            lo, hi = (lvmin_s - 1) * P, lv_s * P
            s01b[si, : hi - lo] = (1 - sm[b]).astype(np.float32)[lo:hi]
        e01 = np.stack(
            [
                np.ascontiguousarray(
                    (1 - em[b]).astype(np.float32).reshape(KB, P).T
                )
                for b in bs
            ]
        )
        in_maps.append(
            {
                "et": et,
                "srct": srct,
                "srcz": np.ascontiguousarray(srcz.astype(np.float32)),
                "en": en,
                "wt": wt,
                "s01b": s01b,
                "e01": e01,
                "ebig": ((1.0 - e01) * np.float32(1e30)).astype(np.float32),
            }
        )
    return in_maps


def kernel(edit_encodings, src_encodings, edit_sent_masks, src_sent_masks, W):
    E = np.ascontiguousarray(np.asarray(edit_encodings, dtype=np.float32))
    S = np.ascontiguousarray(np.asarray(src_encodings, dtype=np.float32))
    em = np.asarray(edit_sent_masks).astype(np.int32)
    sm = np.asarray(src_sent_masks).astype(np.int32)
    W32 = np.ascontiguousarray(np.asarray(W, dtype=np.float32))

    assignment, slots = plan_assignment(em, sm)
    nc = get_nc(slots)
    in_maps = make_in_maps(E, S, em, sm, W32, assignment)
    res = run_bass_kernel_spmd(nc, in_maps, core_ids=list(range(NCORES)))

    edit_ctx = np.empty((B, L, L), np.float32)
    src_ctx = np.empty((B, L, L), np.float32)
    for c in range(NCORES):
        for s in range(PER_CORE):
            b = assignment[c][s]
            edit_ctx[b] = res.results[c]["oe"][s]
            src_ctx[b] = res.results[c]["osr"][s]
    return edit_ctx, src_ctx
